# revision 8
# baseline (speedup 1.0000x reference)
"""DGCNN edge-conv block on 8 Trainium2 NeuronCores.

Sharding: data-parallel over (batch, query-half): core i handles batch i//2,
queries [1024*(i%2)*2 : +2048] of that batch's 4096 points. Each core gets the
full point cloud of its batch (keys) with columns permuted so its own queries
are always columns 0..2047 (SPMD: one program, per-core inputs).

Per-core pipeline (all fp32):
  1. xx[n] = sum_c x[c,n]^2 (ACT square + PE ones-reduce), mxx = -xx/2
  2. A  = w1[:, :1000] @ x   [64, 4096]  (pre-BN "neighbor" features, all keys)
     Bv = w1[:, 1000:] @ x_q [64, 2048]  (pre-BN "center" features, queries)
  3. s/2[q,n] = sum_c x[c,q] x[c,n] - xx[n]/2   (PE, psum-accumulated; the
     -xx/2 row is folded in as a K=1 matmul). Ranking-equivalent to the
     reference's pd (per-row shift by xx[q]/2 and scale by 2 preserve order).
  4. top-8 values+indices per query row (DVE max / max_index); keep top-3.
     Neighbor ORDER is irrelevant downstream (max over k), only the set.
  5. indices -> DRAM in ap_gather's wrapped layout -> SBUF
  6. h1 = relu(bn1(A[:, idx] + Bv))  via gpsimd ap_gather + DVE add + ACT
  7. conv2..conv4 with max-over-k after each (PE + ACT BN-relu + DVE max),
     cat -> conv5 -> out [1024, 2048]
"""

import sys

sys.path.insert(0, "/opt/trn_rl_repo")

import numpy as np

B, C_IN, N, K = 4, 1000, 4096, 3
NQ = 2048          # queries per core
CP = 125           # contraction chunk partitions (1000 = 8*125)
CH = 8             # number of contraction chunks
NT = 512           # key tile (psum bank width in fp32)
NNT = N // NT      # 8 key tiles
QT = 128           # query tile (psum partitions)
NQT = NQ // QT     # 16 query tiles
SEG = 512          # conv-phase query segment
NSEG = NQ // SEG   # 4 segments
EPS = np.float32(1e-5)

_CACHE = {}


def build_nc(finalize=True):
    import concourse.mybir as mybir
    import concourse.tile as tile
    from concourse import bacc

    f32 = mybir.dt.float32
    u16 = mybir.dt.uint16
    i16 = mybir.dt.int16
    Relu = mybir.ActivationFunctionType.Relu

    nc = bacc.Bacc("TRN2", target_bir_lowering=False, debug=False, num_devices=8)

    xb = nc.dram_tensor("xb", [C_IN, N], f32, kind="ExternalInput").ap()
    w1t = nc.dram_tensor("w1t", [C_IN, 128], f32, kind="ExternalInput").ap()
    w2t = nc.dram_tensor("w2t", [64, 128], f32, kind="ExternalInput").ap()
    w3t = nc.dram_tensor("w3t", [128, 256], f32, kind="ExternalInput").ap()
    w4t = nc.dram_tensor("w4t", [256, 512], f32, kind="ExternalInput").ap()
    w5p = nc.dram_tensor("w5p", [128, 8, 1024], f32, kind="ExternalInput").ap()
    sb1 = nc.dram_tensor("sb1", [64, 2], f32, kind="ExternalInput").ap()
    sb2 = nc.dram_tensor("sb2", [128, 2], f32, kind="ExternalInput").ap()
    sb3 = nc.dram_tensor("sb3", [128, 4], f32, kind="ExternalInput").ap()
    sb4 = nc.dram_tensor("sb4", [128, 8], f32, kind="ExternalInput").ap()
    sb5 = nc.dram_tensor("sb5", [128, 16], f32, kind="ExternalInput").ap()
    out = nc.dram_tensor("out", [1024, NQ], f32, kind="ExternalOutput").ap()

    with tile.TileContext(nc) as tc:
        _body(nc, tc, mybir, xb, w1t, w2t, w3t, w4t, w5p,
              sb1, sb2, sb3, sb4, sb5, out, f32, u16, i16, Relu)
    if finalize:
        nc.finalize()
    return nc


def _body(nc, tc, mybir, xb, w1t, w2t, w3t, w4t, w5p,
          sb1, sb2, sb3, sb4, sb5, out, f32, u16, i16, Relu):
    from contextlib import ExitStack

    es = ExitStack()
    with es:
        p_c1 = es.enter_context(tc.tile_pool(name="c1", bufs=1))
        p_work = es.enter_context(tc.tile_pool(name="work", bufs=1))
        p_dram = es.enter_context(tc.tile_pool(name="dram", bufs=1, space="DRAM"))

        # ---- phase-1 constants ----
        w1s = p_c1.tile([CP, CH, 128], f32, tag="w1s")
        nc.sync.dma_start(w1s[:], w1t.rearrange("(c p) m -> p c m", p=CP))
        ones_col = p_c1.tile([128, 1], f32, tag="ones_col")
        nc.vector.memset(ones_col[:], 1.0)
        ones_row = p_c1.tile([1, 128], f32, tag="ones_row")
        nc.vector.memset(ones_row[:], 1.0)

        # ---- persistent working tensors ----
        mxx = p_work.tile([1, N], f32, tag="mxx")          # -xx/2
        A = p_work.tile([64, N], f32, tag="A")             # w1n @ x (pre-BN)
        Bv = p_work.tile([64, NQ], f32, tag="Bv")          # w1c @ x_q (pre-BN)
        w16d = p_dram.tile([16, 3 * NQ // 16], u16, tag="w16d")

        with tc.tile_pool(name="px", bufs=1) as p_x:
            xs = p_x.tile([CP, CH, N], f32, tag="xs")
            nc.sync.dma_start(xs[:], xb.rearrange("(c p) n -> p c n", p=CP))

            # ---- phase A1: xx ----
            with nc.named_scope("xx"):
                with tc.tile_pool(name="sq", bufs=2) as p_sq, \
                     tc.tile_pool(name="psxx", bufs=8, space="PSUM") as p_psxx:
                    psxx = [p_psxx.tile([1, NT], f32, tag="psxx", name=f"psxx{_i}")
                            for _i in range(NNT)]
                    for c in range(CH):
                        for h in range(2):
                            sq = p_sq.tile([CP, N // 2], f32, tag="sq")
                            nc.scalar.square(sq[:], xs[:, c, h * (N // 2):(h + 1) * (N // 2)])
                            for j in range(NNT // 2):
                                nt = h * (NNT // 2) + j
                                nc.tensor.matmul(
                                    psxx[nt][:], ones_col[0:CP, :],
                                    sq[:, j * NT:(j + 1) * NT],
                                    start=(c == 0), stop=(c == CH - 1))
                    for nt in range(NNT):
                        nc.scalar.mul(mxx[:, nt * NT:(nt + 1) * NT], psxx[nt][:], -0.5)

            # ---- phase A2: A and Bv ----
            with nc.named_scope("w1mm"):
                with tc.tile_pool(name="psa", bufs=2, space="PSUM") as p_psa:
                    for nt in range(NNT):
                        pa = p_psa.tile([64, NT], f32, tag="pa")
                        for c in range(CH):
                            nc.tensor.matmul(
                                pa[:], w1s[:, c, 0:64], xs[:, c, nt * NT:(nt + 1) * NT],
                                start=(c == 0), stop=(c == CH - 1))
                        nc.scalar.copy(A[:, nt * NT:(nt + 1) * NT], pa[:])
                    for nt in range(NQ // NT):
                        pb = p_psa.tile([64, NT], f32, tag="pb")
                        for c in range(CH):
                            nc.tensor.matmul(
                                pb[:], w1s[:, c, 64:128], xs[:, c, nt * NT:(nt + 1) * NT],
                                start=(c == 0), stop=(c == CH - 1))
                        nc.scalar.copy(Bv[:, nt * NT:(nt + 1) * NT], pb[:])

            # ---- phase B: distances + top-k ----
            with nc.named_scope("knn"):
                with tc.tile_pool(name="srow", bufs=1) as p_s, \
                     tc.tile_pool(name="m8", bufs=2) as p_m8, \
                     tc.tile_pool(name="pss", bufs=4, space="PSUM") as p_pss:
                    wr = w16d.rearrange("r (kk qt g) -> qt g r kk", kk=3, qt=NQT, g=8)
                    for qt in range(NQT):
                        srow = p_s.tile([QT, N], f32, tag="srow")
                        for nt in range(NNT):
                            ps = p_pss.tile([QT, NT], f32, tag="pss")
                            for c in range(CH):
                                nc.tensor.matmul(
                                    ps[:], xs[:, c, qt * QT:(qt + 1) * QT],
                                    xs[:, c, nt * NT:(nt + 1) * NT],
                                    start=(c == 0), stop=False)
                            nc.tensor.matmul(
                                ps[:], ones_row[:, 0:QT], mxx[:, nt * NT:(nt + 1) * NT],
                                start=False, stop=True)
                            nc.scalar.copy(srow[:, nt * NT:(nt + 1) * NT], ps[:])
                        m8 = p_m8.tile([QT, 8], f32, tag="m8")
                        i8 = p_m8.tile([QT, 8], u16, tag="i8")
                        nc.vector.max(out=m8[:], in_=srow[:])
                        nc.vector.max_index(out=i8[:], in_max=m8[:], in_values=srow[:])
                        nc.sync.dma_start(wr[qt], i8[:, 0:3])

        # ---- phase C/D constants + buffers (reuse the freed x space) ----
        with tc.tile_pool(name="cd", bufs=1) as p_cd:
            w2s = p_cd.tile([64, 128], f32, tag="w2s")
            nc.sync.dma_start(w2s[:], w2t[:])
            w3s = p_cd.tile([128, 256], f32, tag="w3s")
            nc.sync.dma_start(w3s[:], w3t[:])
            w4s = p_cd.tile([128, 2, 512], f32, tag="w4s")
            nc.sync.dma_start(w4s[:], w4t.rearrange("(c p) m -> p c m", p=128))
            w5s = p_cd.tile([128, 8, 1024], f32, tag="w5s")
            nc.sync.dma_start(w5s[:], w5p[:])
            sb1s = p_cd.tile([64, 2], f32, tag="sb1s")
            nc.sync.dma_start(sb1s[:], sb1[:])
            sb2s = p_cd.tile([128, 2], f32, tag="sb2s")
            nc.sync.dma_start(sb2s[:], sb2[:])
            sb3s = p_cd.tile([128, 4], f32, tag="sb3s")
            nc.sync.dma_start(sb3s[:], sb3[:])
            sb4s = p_cd.tile([128, 8], f32, tag="sb4s")
            nc.sync.dma_start(sb4s[:], sb4[:])
            sb5s = p_cd.tile([128, 16], f32, tag="sb5s")
            nc.sync.dma_start(sb5s[:], sb5[:])
            g1 = p_cd.tile([64, 3 * NQ], f32, tag="g1")
            idxw = p_cd.tile([64, 3 * NQ // 16], i16, tag="idxw")

            # ---- phase C: gather + h1 ----
            with nc.named_scope("gather"):
                for g in range(4):
                    nc.sync.dma_start(idxw[16 * g:16 * (g + 1), :], w16d.bitcast(i16))
                from concourse import library_config
                with tc.tile_critical():
                    nc.gpsimd.load_library(library_config.ap_gather)
                    nc.gpsimd.ap_gather(
                        out_ap=g1[:], in_ap=A[:], idxs_ap=idxw[:],
                        channels=64, num_elems=N, d=1, num_idxs=3 * NQ)
                bvb = Bv.unsqueeze(1).to_broadcast([64, 3, NQ])
                g13 = g1.rearrange("p (k q) -> p k q", k=3)
                nc.vector.tensor_add(g13, g13, bvb)
                nc.scalar.activation(g1[:], g1[:], Relu,
                                     bias=sb1s[:, 1:2], scale=sb1s[:, 0:1])
            h1 = g1.rearrange("p (k q) -> p k q", k=3)  # [64, 3, NQ] post-relu

            # ---- phase D: convs ----
            with nc.named_scope("convs"):
                with tc.tile_pool(name="seg", bufs=1) as p_seg, \
                     tc.tile_pool(name="tmp", bufs=2) as p_tmp, \
                     tc.tile_pool(name="osb", bufs=1) as p_osb, \
                     tc.tile_pool(name="psd", bufs=4, space="PSUM") as p_psd:
                    outr = out.rearrange("(c p) n -> p c n", p=128)
                    for seg in range(NSEG):
                        qs = slice(seg * SEG, (seg + 1) * SEG)
                        h2 = p_seg.tile([128, 3, SEG], f32, tag="h2")
                        h3 = p_seg.tile([128, 2, 3, SEG], f32, tag="h3")
                        h4 = p_seg.tile([128, 4, 3, SEG], f32, tag="h4")
                        cat = p_seg.tile([128, 8, SEG], f32, tag="cat")
                        osb = p_osb.tile([128, 8, SEG], f32, tag="osb")
                        nc.vector.memset(cat[64:128, 0, :], 0.0)

                        # conv2 (K=64 -> 128)
                        for kk in range(3):
                            ps2 = p_psd.tile([128, SEG], f32, tag="psd")
                            nc.tensor.matmul(ps2[:], w2s[:], h1[:, kk, qs],
                                             start=True, stop=True)
                            nc.scalar.activation(h2[:, kk, :], ps2[:], Relu,
                                                 bias=sb2s[:, 1:2], scale=sb2s[:, 0:1])
                        # x1 -> cat chunk 0 (64 rows)
                        t1 = p_tmp.tile([64, SEG], f32, tag="t64")
                        nc.vector.tensor_max(t1[:], h1[:, 1, qs], h1[:, 2, qs])
                        nc.vector.tensor_max(cat[0:64, 0, :], t1[:], h1[:, 0, qs])
                        # x2 -> cat chunk 1
                        t2 = p_tmp.tile([128, SEG], f32, tag="t128")
                        nc.vector.tensor_max(t2[:], h2[:, 1, :], h2[:, 2, :])
                        nc.vector.tensor_max(cat[:, 1, :], t2[:], h2[:, 0, :])

                        # conv3 (K=128 -> 256 in 2 chunks)
                        for m in range(2):
                            for kk in range(3):
                                ps3 = p_psd.tile([128, SEG], f32, tag="psd")
                                nc.tensor.matmul(ps3[:], w3s[:, m * 128:(m + 1) * 128],
                                                 h2[:, kk, :], start=True, stop=True)
                                nc.scalar.activation(h3[:, m, kk, :], ps3[:], Relu,
                                                     bias=sb3s[:, 2 + m:3 + m],
                                                     scale=sb3s[:, m:m + 1])
                        # x3 -> cat chunks 2,3
                        for m in range(2):
                            t3 = p_tmp.tile([128, SEG], f32, tag="t128")
                            nc.vector.tensor_max(t3[:], h3[:, m, 1, :], h3[:, m, 2, :])
                            nc.vector.tensor_max(cat[:, 2 + m, :], t3[:], h3[:, m, 0, :])

                        # conv4 (K=256 in 2 chunks -> 512 in 4 chunks)
                        for m in range(4):
                            for kk in range(3):
                                ps4 = p_psd.tile([128, SEG], f32, tag="psd")
                                for c in range(2):
                                    nc.tensor.matmul(
                                        ps4[:], w4s[:, c, m * 128:(m + 1) * 128],
                                        h3[:, c, kk, :], start=(c == 0), stop=(c == 1))
                                nc.scalar.activation(h4[:, m, kk, :], ps4[:], Relu,
                                                     bias=sb4s[:, 4 + m:5 + m],
                                                     scale=sb4s[:, m:m + 1])
                        # x4 -> cat chunks 4..7
                        for m in range(4):
                            t4 = p_tmp.tile([128, SEG], f32, tag="t128")
                            nc.vector.tensor_max(t4[:], h4[:, m, 1, :], h4[:, m, 2, :])
                            nc.vector.tensor_max(cat[:, 4 + m, :], t4[:], h4[:, m, 0, :])

                        # conv5 (K=960 padded to 8*128 -> 1024 in 8 chunks)
                        for m in range(8):
                            ps5 = p_psd.tile([128, SEG], f32, tag="psd")
                            for c in range(8):
                                nc.tensor.matmul(
                                    ps5[:], w5s[:, c, m * 128:(m + 1) * 128],
                                    cat[:, c, :], start=(c == 0), stop=(c == 7))
                            nc.scalar.activation(osb[:, m, :], ps5[:], Relu,
                                                 bias=sb5s[:, 8 + m:9 + m],
                                                 scale=sb5s[:, m:m + 1])
                        nc.sync.dma_start(outr[:, :, qs], osb[:])


def prep_inputs(inputs):
    """Host-side sharding + layout prep. Returns (in_maps, meta)."""
    x = np.ascontiguousarray(inputs["x"], dtype=np.float32)  # [B, C, N]
    shared = {}
    w1 = inputs["w1"].astype(np.float32)
    shared["w1t"] = np.ascontiguousarray(
        np.concatenate([w1[:, :C_IN].T, w1[:, C_IN:].T], axis=1))  # [1000, 128]
    shared["w2t"] = np.ascontiguousarray(inputs["w2"].astype(np.float32).T)
    shared["w3t"] = np.ascontiguousarray(inputs["w3"].astype(np.float32).T)
    shared["w4t"] = np.ascontiguousarray(inputs["w4"].astype(np.float32).T)
    w5t = inputs["w5"].astype(np.float32).T  # [960, 1024]
    w5p = np.zeros((128, 8, 1024), dtype=np.float32)
    w5p[0:64, 0, :] = w5t[0:64]          # x1 block
    w5p[:, 1, :] = w5t[64:192]           # x2
    w5p[:, 2, :] = w5t[192:320]          # x3 lo
    w5p[:, 3, :] = w5t[320:448]          # x3 hi
    for m in range(4):                   # x4
        w5p[:, 4 + m, :] = w5t[448 + 128 * m:448 + 128 * (m + 1)]
    shared["w5p"] = w5p

    def scale_bias(i, cout):
        g = inputs[f"g{i}"].astype(np.float32)
        b = inputs[f"b{i}"].astype(np.float32)
        m = inputs[f"m{i}"].astype(np.float32)
        v = inputs[f"v{i}"].astype(np.float32)
        s = g / np.sqrt(v + EPS)
        return s.astype(np.float32), (b - m * s).astype(np.float32)

    s1, b1 = scale_bias(1, 64)
    shared["sb1"] = np.ascontiguousarray(np.stack([s1, b1], axis=1))
    s2, b2 = scale_bias(2, 128)
    shared["sb2"] = np.ascontiguousarray(np.stack([s2, b2], axis=1))
    s3, b3 = scale_bias(3, 256)
    shared["sb3"] = np.ascontiguousarray(
        np.stack([s3[:128], s3[128:], b3[:128], b3[128:]], axis=1))
    s4, b4 = scale_bias(4, 512)
    shared["sb4"] = np.ascontiguousarray(np.stack(
        [s4[128 * m:128 * (m + 1)] for m in range(4)]
        + [b4[128 * m:128 * (m + 1)] for m in range(4)], axis=1))
    s5, b5 = scale_bias(5, 1024)
    shared["sb5"] = np.ascontiguousarray(np.stack(
        [s5[128 * m:128 * (m + 1)] for m in range(8)]
        + [b5[128 * m:128 * (m + 1)] for m in range(8)], axis=1))

    in_maps = []
    for core in range(8):
        b, half = core // 2, core % 2
        q0 = half * NQ
        # columns: [own queries][other half]
        other0 = NQ - q0  # 2048 if half==0 else 0
        xbp = np.concatenate([x[b][:, q0:q0 + NQ], x[b][:, other0:other0 + NQ]], axis=1)
        m = dict(shared)
        m["xb"] = np.ascontiguousarray(xbp)
        in_maps.append(m)
    return in_maps


def kernel(**inputs):
    from concourse.bass_utils import run_bass_kernel_spmd

    if "nc" not in _CACHE:
        _CACHE["nc"] = build_nc()
    nc = _CACHE["nc"]
    in_maps = prep_inputs(inputs)
    res = run_bass_kernel_spmd(nc, in_maps, core_ids=list(range(8)))
    out = np.empty((B, 1024, N), dtype=np.float32)
    for core in range(8):
        b, half = core // 2, core % 2
        q0 = half * NQ
        out[b, :, q0:q0 + NQ] = res.results[core]["out"]
    return out


# revision 14
# speedup vs baseline: 1.4849x; 1.4849x over previous
"""DGCNN edge-conv block on 8 Trainium2 NeuronCores.

Sharding: data-parallel over (batch, query-half): core i handles batch i//2,
queries [2048*(i%2) : +2048] of that batch's 4096 points. Each core gets the
full point cloud of its batch (keys) with columns permuted so its own queries
are always columns 0..2047 (SPMD: one program, per-core inputs).

Numerics: the KNN scores are computed via a bf16 hi/lo split of x
(x = hi + lo exactly to 2^-18): s/2 = hi.hi + hi.lo + lo.hi - xx/2, all
accumulated in fp32 PSUM. Extra error vs a native fp32 matmul is ~6e-5 -
below fp32's own reduction noise (~3e-4) - so the selected top-3 sets match
the fp32 reference (verified empirically: zero flips on the benchmark input).
The -xx/2 per-key offset is folded into the same PSUM group as a K=3 matmul
of a 3-way bf16 split of the fp32 xx vector (split error ~4e-6). xx itself
is computed in fp32 on the PE (ones-vector reduction of squares).
Conv weights/activations use fp16 (values only, no selection; fp32 PSUM),
final BN+ReLU writes fp32.

Pipeline per core:
  A: load hi/lo; xr=hi+lo; xx (fp32); A=w1n@x, Bv=w1c@x_q (fp16 mm) -> DRAM
  B: per (query-tile, key-tile): 24 bf16 matmuls + xx-fold; top-8 via
     DVE max/max_index; top-3 indices -> DRAM in ap_gather wrapped layout
  C: gather A columns (gpsimd ap_gather), + Bv, BN+ReLU -> h1 (fp16)
  D: conv2..conv4 with max-over-k, cat, conv5 -> out [1024, 2048] fp32
"""

import sys

sys.path.insert(0, "/opt/trn_rl_repo")

import numpy as np

B, C_IN, N, K = 4, 1000, 4096, 3
CPAD = 1024        # padded contraction dim
NQ = 2048          # queries per core
CP = 128           # contraction chunk partitions
CH = 8             # number of contraction chunks
NT = 512           # key tile (psum bank width in fp32)
NNT = N // NT      # 8 key tiles
QT = 128           # query tile (psum partitions)
NQT = NQ // QT     # 16 query tiles
SEG = 512          # conv-phase query segment
NSEG = NQ // SEG   # 4 segments
EPS = np.float32(1e-5)

_CACHE = {}


def build_nc(finalize=True):
    import concourse.mybir as mybir
    import concourse.tile as tile
    from concourse import bacc

    f32 = mybir.dt.float32
    f16 = mybir.dt.float16
    bf16 = mybir.dt.bfloat16
    u16 = mybir.dt.uint16
    i16 = mybir.dt.int16
    Relu = mybir.ActivationFunctionType.Relu

    nc = bacc.Bacc("TRN2", target_bir_lowering=False, debug=False, num_devices=8)

    xh = nc.dram_tensor("xh", [CPAD, N], bf16, kind="ExternalInput").ap()
    xl = nc.dram_tensor("xl", [CPAD, N], bf16, kind="ExternalInput").ap()
    w1t = nc.dram_tensor("w1t", [CPAD, 128], f16, kind="ExternalInput").ap()
    w2t = nc.dram_tensor("w2t", [64, 128], f16, kind="ExternalInput").ap()
    w3t = nc.dram_tensor("w3t", [128, 256], f16, kind="ExternalInput").ap()
    w4t = nc.dram_tensor("w4t", [256, 512], f16, kind="ExternalInput").ap()
    w5p = nc.dram_tensor("w5p", [128, 8, 1024], f16, kind="ExternalInput").ap()
    sb1 = nc.dram_tensor("sb1", [64, 2], f32, kind="ExternalInput").ap()
    sb2 = nc.dram_tensor("sb2", [128, 2], f32, kind="ExternalInput").ap()
    sb3 = nc.dram_tensor("sb3", [128, 4], f32, kind="ExternalInput").ap()
    sb4 = nc.dram_tensor("sb4", [128, 8], f32, kind="ExternalInput").ap()
    sb5 = nc.dram_tensor("sb5", [128, 16], f32, kind="ExternalInput").ap()
    out = nc.dram_tensor("out", [1024, NQ], f32, kind="ExternalOutput").ap()

    with tile.TileContext(nc) as tc:
        _body(nc, tc, mybir, xh, xl, w1t, w2t, w3t, w4t, w5p,
              sb1, sb2, sb3, sb4, sb5, out, f32, f16, bf16, u16, i16, Relu)
    if finalize:
        nc.finalize()
    return nc


def _body(nc, tc, mybir, xh, xl, w1t, w2t, w3t, w4t, w5p,
          sb1, sb2, sb3, sb4, sb5, out, f32, f16, bf16, u16, i16, Relu):
    from contextlib import ExitStack
    from concourse import library_config

    es = ExitStack()
    with es:
        p_c1 = es.enter_context(tc.tile_pool(name="c1", bufs=1))
        p_dram = es.enter_context(tc.tile_pool(name="dram", bufs=1, space="DRAM"))

        # gpsimd library for the phase-C gather: load it up front so the
        # ~170us ucode DMA + drain overlaps phases A/B instead of stalling C.
        nc.gpsimd.load_library(library_config.ap_gather)

        # ---- phase-1 constants ----
        w1s = p_c1.tile([CP, CH, 128], f16, tag="w1s")
        nc.sync.dma_start(w1s[:], w1t.rearrange("(c p) m -> p c m", p=CP))
        ones_col = p_c1.tile([128, 1], f32, tag="ones_col")
        nc.vector.memset(ones_col[:], 1.0)
        ones3 = p_c1.tile([3, 128], bf16, tag="ones3")
        nc.vector.memset(ones3[:], 1.0)

        # DRAM scratch
        w16d = p_dram.tile([16, 3 * NQ // 16], u16, tag="w16d")
        Ad = p_dram.tile([64, N], f32, tag="Ad")
        Bvd = p_dram.tile([64, NQ], f32, tag="Bvd")

        with tc.tile_pool(name="bx", bufs=1) as p_bx:
            xhs = p_bx.tile([CP, CH, N], bf16, tag="xhs")
            xls = p_bx.tile([CP, CH, N], bf16, tag="xls")
            xhr = xh.rearrange("(c p) n -> p c n", p=CP)
            xlr = xl.rearrange("(c p) n -> p c n", p=CP)
            for c in range(CH):
                nc.sync.dma_start(xhs[:, c, :], xhr[:, c, :])
                nc.sync.dma_start(xls[:, c, :], xlr[:, c, :])
            mxxs = p_bx.tile([3, N], bf16, tag="mxxs")

            with tc.tile_pool(name="ms", bufs=1) as p_s, \
                 tc.tile_pool(name="m8", bufs=2) as p_m8:
                # ---- phase A: xx, A, Bv (two streamed passes) ----
                with nc.named_scope("prep"):
                    # pass 1: xx via fp32 ones-reduction of squares
                    with tc.tile_pool(name="pa1", bufs=1) as p_a1, \
                         tc.tile_pool(name="psxx", bufs=8, space="PSUM") as p_psxx:
                        mxx = p_a1.tile([1, N], f32, tag="mxx")
                        psxx = [p_psxx.tile([1, NT], f32, tag="psxx",
                                            name=f"psxx{_i}") for _i in range(NNT)]
                        for c in range(CH):
                            t = p_a1.tile([CP, N], f32, tag="xr", name=f"xr{c}")
                            nc.vector.tensor_add(t[:], xhs[:, c, :], xls[:, c, :])
                            nc.scalar.square(t[:], t[:])
                            for nt in range(NNT):
                                nc.tensor.matmul(
                                    psxx[nt][:], ones_col[:, :],
                                    t[:, nt * NT:(nt + 1) * NT],
                                    start=(c == 0), stop=(c == CH - 1))
                        for nt in range(NNT):
                            nc.scalar.mul(mxx[:, nt * NT:(nt + 1) * NT],
                                          psxx[nt][:], -0.5)
                        # 3-way bf16 split of mxx (split error ~4e-6);
                        # staged at partition 0, DMA'd to rows 1/2 (engines
                        # can only address start partitions 0/32/64/96)
                        msp = p_a1.tile([1, N], bf16, tag="msp")
                        for j in range(3):
                            nc.scalar.copy(msp[:], mxx[:])
                            nc.sync.dma_start(mxxs[j:j + 1, :], msp[:])
                            if j < 2:
                                nc.vector.tensor_sub(mxx[:], mxx[:], msp[:])
                    # pass 2: A / Bv in fp16, c-outer with 8 resident psum banks
                    with tc.tile_pool(name="pa2", bufs=2) as p_a2, \
                         tc.tile_pool(name="psa", bufs=8, space="PSUM") as p_psa:
                        pas = [p_psa.tile([128, NT], f32, tag="pa",
                                          name=f"pa{_i}") for _i in range(NNT)]
                        for c in range(CH):
                            x16 = p_a2.tile([CP, N], f16, tag="x16", name=f"x16_{c}")
                            nc.vector.tensor_add(x16[:], xhs[:, c, :], xls[:, c, :])
                            for nt in range(NNT):
                                # query tiles get both halves of w1 (A and Bv
                                # stacked); key-only tiles just the A half
                                mw = 128 if nt < NQ // NT else 64
                                nc.tensor.matmul(
                                    pas[nt][0:mw, :], w1s[:, c, 0:mw],
                                    x16[:, nt * NT:(nt + 1) * NT],
                                    start=(c == 0), stop=(c == CH - 1))
                        for nt in range(NNT):
                            st = p_a2.tile([128, NT], f32, tag="stg", name=f"stg{nt}")
                            nc.scalar.copy(st[0:64, :], pas[nt][0:64, :])
                            nc.sync.dma_start(Ad[:, nt * NT:(nt + 1) * NT],
                                              st[0:64, :])
                            if nt < NQ // NT:
                                nc.scalar.copy(st[64:128, :], pas[nt][64:128, :])
                                nc.sync.dma_start(Bvd[:, nt * NT:(nt + 1) * NT],
                                                  st[64:128, :])

                # ---- phase B: distances + top-k ----
                # s/2 = hi.hi + hi.lo + lo.hi + ones3.mxxs, fp32 PSUM accumulate
                with nc.named_scope("knn"):
                    with tc.tile_pool(name="pss", bufs=6, space="PSUM") as p_pss:
                        wr = w16d.rearrange("r (kk qt g) -> qt g r kk",
                                            kk=3, qt=NQT, g=8)
                        for qt in range(NQT):
                            qs = slice(qt * QT, (qt + 1) * QT)
                            srow = p_s.tile([QT, N], f32, tag="srow")
                            for nt in range(NNT):
                                ns = slice(nt * NT, (nt + 1) * NT)
                                ps = p_pss.tile([QT, NT], f32, tag="pss")
                                for c in range(CH):
                                    nc.tensor.matmul(ps[:], xhs[:, c, qs],
                                                     xhs[:, c, ns],
                                                     start=(c == 0), stop=False)
                                    nc.tensor.matmul(ps[:], xhs[:, c, qs],
                                                     xls[:, c, ns],
                                                     start=False, stop=False)
                                for c in range(CH):
                                    nc.tensor.matmul(ps[:], xls[:, c, qs],
                                                     xhs[:, c, ns],
                                                     start=False, stop=False)
                                nc.tensor.matmul(ps[:], ones3[:, 0:QT], mxxs[:, ns],
                                                 start=False, stop=True)
                                nc.scalar.copy(srow[:, ns], ps[:])
                            m8 = p_m8.tile([QT, 8], f32, tag="m8")
                            i8 = p_m8.tile([QT, 8], u16, tag="i8")
                            nc.vector.max(out=m8[:], in_=srow[:])
                            nc.vector.max_index(out=i8[:], in_max=m8[:],
                                                in_values=srow[:])
                            nc.sync.dma_start(wr[qt], i8[:, 0:3])

        # ---- phase C/D (reuse the freed x space) ----
        with tc.tile_pool(name="cd", bufs=1) as p_cd:
            A = p_cd.tile([64, N], f32, tag="A")
            nc.sync.dma_start(A[:], Ad[:])
            Bv = p_cd.tile([64, NQ], f32, tag="Bv")
            nc.sync.dma_start(Bv[:], Bvd[:])
            w2s = p_cd.tile([64, 128], f16, tag="w2s")
            nc.sync.dma_start(w2s[:], w2t[:])
            w3s = p_cd.tile([128, 256], f16, tag="w3s")
            nc.sync.dma_start(w3s[:], w3t[:])
            w4s = p_cd.tile([128, 2, 512], f16, tag="w4s")
            nc.sync.dma_start(w4s[:], w4t.rearrange("(c p) m -> p c m", p=128))
            w5s = p_cd.tile([128, 8, 1024], f16, tag="w5s")
            nc.sync.dma_start(w5s[:], w5p[:])
            sb1s = p_cd.tile([64, 2], f32, tag="sb1s")
            nc.sync.dma_start(sb1s[:], sb1[:])
            sb2s = p_cd.tile([128, 2], f32, tag="sb2s")
            nc.sync.dma_start(sb2s[:], sb2[:])
            sb3s = p_cd.tile([128, 4], f32, tag="sb3s")
            nc.sync.dma_start(sb3s[:], sb3[:])
            sb4s = p_cd.tile([128, 8], f32, tag="sb4s")
            nc.sync.dma_start(sb4s[:], sb4[:])
            sb5s = p_cd.tile([128, 16], f32, tag="sb5s")
            nc.sync.dma_start(sb5s[:], sb5[:])
            g1 = p_cd.tile([64, 3 * NQ], f32, tag="g1")
            h1f = p_cd.tile([64, 3 * NQ], f16, tag="h1f")
            idxw = p_cd.tile([64, 3 * NQ // 16], i16, tag="idxw")

            # ---- phase C: gather + h1 ----
            with nc.named_scope("gather"):
                for g in range(4):
                    nc.sync.dma_start(idxw[16 * g:16 * (g + 1), :], w16d.bitcast(i16))
                nc.gpsimd.ap_gather(
                    out_ap=g1[:], in_ap=A[:], idxs_ap=idxw[:],
                    channels=64, num_elems=N, d=1, num_idxs=3 * NQ)
                bvb = Bv.unsqueeze(1).to_broadcast([64, 3, NQ])
                g13 = g1.rearrange("p (k q) -> p k q", k=3)
                nc.vector.tensor_add(g13, g13, bvb)
                nc.scalar.activation(h1f[:], g1[:], Relu,
                                     bias=sb1s[:, 1:2], scale=sb1s[:, 0:1])
            h1 = h1f.rearrange("p (k q) -> p k q", k=3)  # [64, 3, NQ] fp16

            # ---- phase D: convs (fp16 weights/acts, fp32 psum) ----
            with nc.named_scope("convs"):
                with tc.tile_pool(name="seg", bufs=2) as p_seg, \
                     tc.tile_pool(name="tmp", bufs=2) as p_tmp, \
                     tc.tile_pool(name="osb", bufs=2) as p_osb, \
                     tc.tile_pool(name="psd", bufs=4, space="PSUM") as p_psd:
                    outr = out.rearrange("(c p) n -> p c n", p=128)
                    for seg in range(NSEG):
                        qs = slice(seg * SEG, (seg + 1) * SEG)
                        h2 = p_seg.tile([128, 3, SEG], f16, tag="h2")
                        h3 = p_seg.tile([128, 2, 3, SEG], f16, tag="h3")
                        h4 = p_seg.tile([128, 4, 3, SEG], f16, tag="h4")
                        cat = p_seg.tile([128, 8, SEG], f16, tag="cat")
                        osb = p_osb.tile([128, 8, SEG], f32, tag="osb")
                        nc.vector.memset(cat[64:128, 0, :], 0.0)

                        # conv2 (K=64 -> 128)
                        for kk in range(3):
                            ps2 = p_psd.tile([128, SEG], f32, tag="psd")
                            nc.tensor.matmul(ps2[:], w2s[:], h1[:, kk, qs],
                                             start=True, stop=True)
                            nc.scalar.activation(h2[:, kk, :], ps2[:], Relu,
                                                 bias=sb2s[:, 1:2], scale=sb2s[:, 0:1])
                        # x1 -> cat chunk 0 (64 rows)
                        t1 = p_tmp.tile([64, SEG], f16, tag="t64")
                        nc.vector.tensor_max(t1[:], h1[:, 1, qs], h1[:, 2, qs])
                        nc.vector.tensor_max(cat[0:64, 0, :], t1[:], h1[:, 0, qs])
                        # x2 -> cat chunk 1
                        t2 = p_tmp.tile([128, SEG], f16, tag="t128")
                        nc.vector.tensor_max(t2[:], h2[:, 1, :], h2[:, 2, :])
                        nc.vector.tensor_max(cat[:, 1, :], t2[:], h2[:, 0, :])

                        # conv3 (K=128 -> 256 in 2 chunks)
                        for m in range(2):
                            for kk in range(3):
                                ps3 = p_psd.tile([128, SEG], f32, tag="psd")
                                nc.tensor.matmul(ps3[:], w3s[:, m * 128:(m + 1) * 128],
                                                 h2[:, kk, :], start=True, stop=True)
                                nc.scalar.activation(h3[:, m, kk, :], ps3[:], Relu,
                                                     bias=sb3s[:, 2 + m:3 + m],
                                                     scale=sb3s[:, m:m + 1])
                        # x3 -> cat chunks 2,3
                        for m in range(2):
                            t3 = p_tmp.tile([128, SEG], f16, tag="t128")
                            nc.vector.tensor_max(t3[:], h3[:, m, 1, :], h3[:, m, 2, :])
                            nc.vector.tensor_max(cat[:, 2 + m, :], t3[:], h3[:, m, 0, :])

                        # conv4 (K=256 in 2 chunks -> 512 in 4 chunks)
                        for m in range(4):
                            for kk in range(3):
                                ps4 = p_psd.tile([128, SEG], f32, tag="psd")
                                for c in range(2):
                                    nc.tensor.matmul(
                                        ps4[:], w4s[:, c, m * 128:(m + 1) * 128],
                                        h3[:, c, kk, :], start=(c == 0), stop=(c == 1))
                                nc.scalar.activation(h4[:, m, kk, :], ps4[:], Relu,
                                                     bias=sb4s[:, 4 + m:5 + m],
                                                     scale=sb4s[:, m:m + 1])
                        # x4 -> cat chunks 4..7
                        for m in range(4):
                            t4 = p_tmp.tile([128, SEG], f16, tag="t128")
                            nc.vector.tensor_max(t4[:], h4[:, m, 1, :], h4[:, m, 2, :])
                            nc.vector.tensor_max(cat[:, 4 + m, :], t4[:], h4[:, m, 0, :])

                        # conv5 (K=960 padded to 8*128 -> 1024 in 8 chunks)
                        for m in range(8):
                            ps5 = p_psd.tile([128, SEG], f32, tag="psd")
                            for c in range(8):
                                nc.tensor.matmul(
                                    ps5[:], w5s[:, c, m * 128:(m + 1) * 128],
                                    cat[:, c, :], start=(c == 0), stop=(c == 7))
                            nc.scalar.activation(osb[:, m, :], ps5[:], Relu,
                                                 bias=sb5s[:, 8 + m:9 + m],
                                                 scale=sb5s[:, m:m + 1])
                        nc.sync.dma_start(outr[:, :, qs], osb[:])


def prep_inputs(inputs):
    """Host-side sharding + layout/precision prep. Returns per-core in_maps."""
    import ml_dtypes

    x = np.ascontiguousarray(inputs["x"], dtype=np.float32)  # [B, C, N]
    shared = {}
    w1 = inputs["w1"].astype(np.float32)
    w1p = np.zeros((CPAD, 128), dtype=np.float16)
    w1p[:C_IN, 0:64] = w1[:, :C_IN].T.astype(np.float16)
    w1p[:C_IN, 64:128] = w1[:, C_IN:].T.astype(np.float16)
    shared["w1t"] = w1p
    shared["w2t"] = np.ascontiguousarray(inputs["w2"].T.astype(np.float16))
    shared["w3t"] = np.ascontiguousarray(inputs["w3"].T.astype(np.float16))
    shared["w4t"] = np.ascontiguousarray(inputs["w4"].T.astype(np.float16))
    w5t = inputs["w5"].astype(np.float32).T  # [960, 1024]
    w5p = np.zeros((128, 8, 1024), dtype=np.float16)
    w5p[0:64, 0, :] = w5t[0:64]          # x1 block
    w5p[:, 1, :] = w5t[64:192]           # x2
    w5p[:, 2, :] = w5t[192:320]          # x3 lo
    w5p[:, 3, :] = w5t[320:448]          # x3 hi
    for m in range(4):                   # x4
        w5p[:, 4 + m, :] = w5t[448 + 128 * m:448 + 128 * (m + 1)]
    shared["w5p"] = w5p

    def scale_bias(i):
        g = inputs[f"g{i}"].astype(np.float32)
        b = inputs[f"b{i}"].astype(np.float32)
        m = inputs[f"m{i}"].astype(np.float32)
        v = inputs[f"v{i}"].astype(np.float32)
        s = g / np.sqrt(v + EPS)
        return s.astype(np.float32), (b - m * s).astype(np.float32)

    s1, b1 = scale_bias(1)
    shared["sb1"] = np.ascontiguousarray(np.stack([s1, b1], axis=1))
    s2, b2 = scale_bias(2)
    shared["sb2"] = np.ascontiguousarray(np.stack([s2, b2], axis=1))
    s3, b3 = scale_bias(3)
    shared["sb3"] = np.ascontiguousarray(
        np.stack([s3[:128], s3[128:], b3[:128], b3[128:]], axis=1))
    s4, b4 = scale_bias(4)
    shared["sb4"] = np.ascontiguousarray(np.stack(
        [s4[128 * m:128 * (m + 1)] for m in range(4)]
        + [b4[128 * m:128 * (m + 1)] for m in range(4)], axis=1))
    s5, b5 = scale_bias(5)
    shared["sb5"] = np.ascontiguousarray(np.stack(
        [s5[128 * m:128 * (m + 1)] for m in range(8)]
        + [b5[128 * m:128 * (m + 1)] for m in range(8)], axis=1))

    in_maps = []
    for core in range(8):
        b, half = core // 2, core % 2
        q0 = half * NQ
        other0 = NQ - q0  # 2048 if half==0 else 0
        xbp = np.concatenate([x[b][:, q0:q0 + NQ], x[b][:, other0:other0 + NQ]],
                             axis=1)  # [1000, 4096], own queries first
        xpad = np.zeros((CPAD, N), dtype=np.float32)
        xpad[:C_IN] = xbp
        hi = xpad.astype(ml_dtypes.bfloat16)
        lo = (xpad - hi.astype(np.float32)).astype(ml_dtypes.bfloat16)
        m = dict(shared)
        m["xh"] = np.ascontiguousarray(hi)
        m["xl"] = np.ascontiguousarray(lo)
        in_maps.append(m)
    return in_maps


def kernel(**inputs):
    from concourse.bass_utils import run_bass_kernel_spmd

    if "nc" not in _CACHE:
        _CACHE["nc"] = build_nc()
    nc = _CACHE["nc"]
    in_maps = prep_inputs(inputs)
    res = run_bass_kernel_spmd(nc, in_maps, core_ids=list(range(8)))
    out = np.empty((B, 1024, N), dtype=np.float32)
    for core in range(8):
        b, half = core // 2, core % 2
        q0 = half * NQ
        out[b, :, q0:q0 + NQ] = res.results[core]["out"]
    return out


# revision 16
# speedup vs baseline: 1.5406x; 1.0375x over previous
"""DGCNN edge-conv block on 8 Trainium2 NeuronCores.

Sharding: data-parallel over (batch, query-half): core i handles batch i//2,
queries [2048*(i%2) : +2048] of that batch's 4096 points. Each core gets the
full point cloud of its batch (keys) with columns permuted so its own queries
are always columns 0..2047 (SPMD: one program, per-core inputs).

Numerics: the KNN scores are computed via a bf16 hi/lo split of x
(x = hi + lo exactly to 2^-18): s/2 = hi.hi + hi.lo + lo.hi - xx/2, all
accumulated in fp32 PSUM. Extra error vs a native fp32 matmul is ~6e-5 -
below fp32's own reduction noise (~3e-4) - so the selected top-3 sets match
the fp32 reference (verified empirically: zero flips on the benchmark input).
The -xx/2 per-key offset is folded into the same PSUM group as a K=3 matmul
of a 3-way bf16 split of the fp32 xx vector (split error ~4e-6). xx itself
is computed in fp32 on the PE (ones-vector reduction of squares).
Conv weights/activations use fp16 (values only, no selection; fp32 PSUM),
final BN+ReLU writes fp32.

Pipeline per core:
  A: load hi/lo; xr=hi+lo; xx (fp32); A=w1n@x, Bv=w1c@x_q (fp16 mm) -> DRAM
  B: per (query-tile, key-tile): 24 bf16 matmuls + xx-fold; top-8 via
     DVE max/max_index; top-3 indices -> DRAM in ap_gather wrapped layout
  C: gather A columns (gpsimd ap_gather), + Bv, BN+ReLU -> h1 (fp16)
  D: conv2..conv4 with max-over-k, cat, conv5 -> out [1024, 2048] fp32
"""

import sys

sys.path.insert(0, "/opt/trn_rl_repo")

import numpy as np

B, C_IN, N, K = 4, 1000, 4096, 3
CPAD = 1024        # padded contraction dim
NQ = 2048          # queries per core
CP = 128           # contraction chunk partitions
CH = 8             # number of contraction chunks
NT = 512           # key tile (psum bank width in fp32)
NNT = N // NT      # 8 key tiles
QT = 128           # query tile (psum partitions)
NQT = NQ // QT     # 16 query tiles
SEG = 512          # conv-phase query segment
NSEG = NQ // SEG   # 4 segments
EPS = np.float32(1e-5)

_CACHE = {}


def build_nc(finalize=True):
    import concourse.mybir as mybir
    import concourse.tile as tile
    from concourse import bacc

    f32 = mybir.dt.float32
    f16 = mybir.dt.float16
    bf16 = mybir.dt.bfloat16
    u16 = mybir.dt.uint16
    i16 = mybir.dt.int16
    Relu = mybir.ActivationFunctionType.Relu

    nc = bacc.Bacc("TRN2", target_bir_lowering=False, debug=False, num_devices=8)

    xh = nc.dram_tensor("xh", [CPAD, N], bf16, kind="ExternalInput").ap()
    xl = nc.dram_tensor("xl", [CPAD, N], bf16, kind="ExternalInput").ap()
    w1t = nc.dram_tensor("w1t", [CPAD, 128], f16, kind="ExternalInput").ap()
    w2t = nc.dram_tensor("w2t", [64, 128], f16, kind="ExternalInput").ap()
    w3t = nc.dram_tensor("w3t", [128, 256], f16, kind="ExternalInput").ap()
    w4t = nc.dram_tensor("w4t", [256, 512], f16, kind="ExternalInput").ap()
    w5p = nc.dram_tensor("w5p", [128, 8, 1024], f16, kind="ExternalInput").ap()
    sb1 = nc.dram_tensor("sb1", [64, 2], f32, kind="ExternalInput").ap()
    sb2 = nc.dram_tensor("sb2", [128, 2], f32, kind="ExternalInput").ap()
    sb3 = nc.dram_tensor("sb3", [128, 4], f32, kind="ExternalInput").ap()
    sb4 = nc.dram_tensor("sb4", [128, 8], f32, kind="ExternalInput").ap()
    sb5 = nc.dram_tensor("sb5", [128, 16], f32, kind="ExternalInput").ap()
    out = nc.dram_tensor("out", [1024, NQ], f32, kind="ExternalOutput").ap()

    with tile.TileContext(nc) as tc:
        _body(nc, tc, mybir, xh, xl, w1t, w2t, w3t, w4t, w5p,
              sb1, sb2, sb3, sb4, sb5, out, f32, f16, bf16, u16, i16, Relu)
    if finalize:
        nc.finalize()
    return nc


def _body(nc, tc, mybir, xh, xl, w1t, w2t, w3t, w4t, w5p,
          sb1, sb2, sb3, sb4, sb5, out, f32, f16, bf16, u16, i16, Relu):
    from contextlib import ExitStack
    from concourse import library_config

    es = ExitStack()
    with es:
        p_c1 = es.enter_context(tc.tile_pool(name="c1", bufs=1))
        p_dram = es.enter_context(tc.tile_pool(name="dram", bufs=1, space="DRAM"))

        # gpsimd library for the phase-C gather: load it up front so the
        # ~170us ucode DMA + drain overlaps phases A/B instead of stalling C.
        nc.gpsimd.load_library(library_config.ap_gather)
        # dummy gather: the library ucode DMA (~170us) happens at the first
        # Pool-engine drain after LOAD_LIB; trigger it now so it overlaps
        # phases A/B instead of stalling the real gather.
        dmy = p_c1.tile([64, 16], f32, tag="dmy")
        dmys = p_c1.tile([64, 4], f32, tag="dmys")
        dmyi = p_c1.tile([64, 1], i16, tag="dmyi")
        nc.vector.memset(dmys[:], 0.0)
        nc.vector.memset(dmyi[:], 0)
        nc.gpsimd.ap_gather(out_ap=dmy[:], in_ap=dmys[:], idxs_ap=dmyi[:],
                            channels=64, num_elems=4, d=1, num_idxs=16)

        # ---- phase-1 constants ----
        w1s = p_c1.tile([CP, CH, 128], f16, tag="w1s")
        nc.sync.dma_start(w1s[:], w1t.rearrange("(c p) m -> p c m", p=CP))
        ones_col = p_c1.tile([128, 1], f32, tag="ones_col")
        nc.vector.memset(ones_col[:], 1.0)
        ones3 = p_c1.tile([3, 128], bf16, tag="ones3")
        nc.vector.memset(ones3[:], 1.0)

        # DRAM scratch
        w16d = p_dram.tile([16, 3 * NQ // 16], u16, tag="w16d")
        Ad = p_dram.tile([64, N], f32, tag="Ad")
        Bvd = p_dram.tile([64, NQ], f32, tag="Bvd")

        with tc.tile_pool(name="bx", bufs=1) as p_bx:
            xhs = p_bx.tile([CP, CH, N], bf16, tag="xhs")
            xls = p_bx.tile([CP, CH, N], bf16, tag="xls")
            xhr = xh.rearrange("(c p) n -> p c n", p=CP)
            xlr = xl.rearrange("(c p) n -> p c n", p=CP)
            for c in range(CH):
                nc.sync.dma_start(xhs[:, c, :], xhr[:, c, :])
                nc.sync.dma_start(xls[:, c, :], xlr[:, c, :])
            mxxs = p_bx.tile([3, N], bf16, tag="mxxs")

            with tc.tile_pool(name="ms", bufs=1) as p_s, \
                 tc.tile_pool(name="m8", bufs=2) as p_m8:
                # ---- phase A: xx, A, Bv (two streamed passes) ----
                with nc.named_scope("prep"):
                    # pass 1: xx via fp32 ones-reduction of squares
                    with tc.tile_pool(name="pa1", bufs=1) as p_a1, \
                         tc.tile_pool(name="psxx", bufs=8, space="PSUM") as p_psxx:
                        mxx = p_a1.tile([1, N], f32, tag="mxx")
                        psxx = [p_psxx.tile([1, NT], f32, tag="psxx",
                                            name=f"psxx{_i}") for _i in range(NNT)]
                        for c in range(CH):
                            t = p_a1.tile([CP, N], f32, tag="xr", name=f"xr{c}")
                            nc.vector.tensor_add(t[:], xhs[:, c, :], xls[:, c, :])
                            nc.scalar.square(t[:], t[:])
                            for nt in range(NNT):
                                nc.tensor.matmul(
                                    psxx[nt][:], ones_col[:, :],
                                    t[:, nt * NT:(nt + 1) * NT],
                                    start=(c == 0), stop=(c == CH - 1))
                        for nt in range(NNT):
                            nc.scalar.mul(mxx[:, nt * NT:(nt + 1) * NT],
                                          psxx[nt][:], -0.5)
                        # 3-way bf16 split of mxx (split error ~4e-6);
                        # staged at partition 0, DMA'd to rows 1/2 (engines
                        # can only address start partitions 0/32/64/96)
                        msp = p_a1.tile([1, N], bf16, tag="msp")
                        for j in range(3):
                            nc.scalar.copy(msp[:], mxx[:])
                            nc.sync.dma_start(mxxs[j:j + 1, :], msp[:])
                            if j < 2:
                                nc.vector.tensor_sub(mxx[:], mxx[:], msp[:])
                # phases A2 + B share the PSUM space: pss (6 banks) for the
                # distance tiles, psa (2 banks) for the A/Bv accumulation;
                # the scheduler interleaves both mm streams on the PE.
                with tc.tile_pool(name="pss", bufs=6, space="PSUM") as p_pss, \
                     tc.tile_pool(name="pa2", bufs=2) as p_a2:
                    with tc.tile_pool(name="psa", bufs=2, space="PSUM") as p_psa:
                        for nt in range(NNT):
                            # query tiles get both halves of w1 (A and Bv
                            # stacked); key-only tiles just the A half
                            mw = 128 if nt < NQ // NT else 64
                            pa = p_psa.tile([128, NT], f32, tag="pa")
                            for c in range(CH):
                                x16 = p_a2.tile([CP, NT], f16, tag="x16")
                                nc.vector.tensor_add(
                                    x16[:], xhs[:, c, nt * NT:(nt + 1) * NT],
                                    xls[:, c, nt * NT:(nt + 1) * NT])
                                nc.tensor.matmul(
                                    pa[0:mw, :], w1s[:, c, 0:mw], x16[:],
                                    start=(c == 0), stop=(c == CH - 1))
                            st = p_a2.tile([128, NT], f32, tag="stg")
                            nc.scalar.copy(st[0:64, :], pa[0:64, :])
                            nc.sync.dma_start(Ad[:, nt * NT:(nt + 1) * NT],
                                              st[0:64, :])
                            if nt < NQ // NT:
                                nc.scalar.copy(st[64:128, :], pa[64:128, :])
                                nc.sync.dma_start(Bvd[:, nt * NT:(nt + 1) * NT],
                                                  st[64:128, :])

                    # ---- phase B: distances + top-k ----
                    # s/2 = hi.hi + hi.lo + lo.hi + ones3.mxxs in fp32 PSUM
                    with nc.named_scope("knn"):
                        wr = w16d.rearrange("r (kk qt g) -> qt g r kk",
                                            kk=3, qt=NQT, g=8)
                        for qt in range(NQT):
                            qs = slice(qt * QT, (qt + 1) * QT)
                            srow = p_s.tile([QT, N], f32, tag="srow")
                            for nt in range(NNT):
                                ns = slice(nt * NT, (nt + 1) * NT)
                                ps = p_pss.tile([QT, NT], f32, tag="pss")
                                for c in range(CH):
                                    nc.tensor.matmul(ps[:], xhs[:, c, qs],
                                                     xhs[:, c, ns],
                                                     start=(c == 0), stop=False)
                                    nc.tensor.matmul(ps[:], xhs[:, c, qs],
                                                     xls[:, c, ns],
                                                     start=False, stop=False)
                                for c in range(CH):
                                    nc.tensor.matmul(ps[:], xls[:, c, qs],
                                                     xhs[:, c, ns],
                                                     start=False, stop=False)
                                nc.tensor.matmul(ps[:], ones3[:, 0:QT], mxxs[:, ns],
                                                 start=False, stop=True)
                                nc.scalar.copy(srow[:, ns], ps[:])
                            m8 = p_m8.tile([QT, 8], f32, tag="m8")
                            i8 = p_m8.tile([QT, 8], u16, tag="i8")
                            nc.vector.max(out=m8[:], in_=srow[:])
                            nc.vector.max_index(out=i8[:], in_max=m8[:],
                                                in_values=srow[:])
                            nc.sync.dma_start(wr[qt], i8[:, 0:3])

        # ---- phase C/D (reuse the freed x space) ----
        with tc.tile_pool(name="cd", bufs=1) as p_cd:
            A = p_cd.tile([64, N], f32, tag="A")
            nc.sync.dma_start(A[:], Ad[:])
            Bv = p_cd.tile([64, NQ], f32, tag="Bv")
            nc.sync.dma_start(Bv[:], Bvd[:])
            w2s = p_cd.tile([64, 128], f16, tag="w2s")
            nc.sync.dma_start(w2s[:], w2t[:])
            w3s = p_cd.tile([128, 256], f16, tag="w3s")
            nc.sync.dma_start(w3s[:], w3t[:])
            w4s = p_cd.tile([128, 2, 512], f16, tag="w4s")
            nc.sync.dma_start(w4s[:], w4t.rearrange("(c p) m -> p c m", p=128))
            w5s = p_cd.tile([128, 8, 1024], f16, tag="w5s")
            nc.sync.dma_start(w5s[:], w5p[:])
            sb1s = p_cd.tile([64, 2], f32, tag="sb1s")
            nc.sync.dma_start(sb1s[:], sb1[:])
            sb2s = p_cd.tile([128, 2], f32, tag="sb2s")
            nc.sync.dma_start(sb2s[:], sb2[:])
            sb3s = p_cd.tile([128, 4], f32, tag="sb3s")
            nc.sync.dma_start(sb3s[:], sb3[:])
            sb4s = p_cd.tile([128, 8], f32, tag="sb4s")
            nc.sync.dma_start(sb4s[:], sb4[:])
            sb5s = p_cd.tile([128, 16], f32, tag="sb5s")
            nc.sync.dma_start(sb5s[:], sb5[:])
            g1 = p_cd.tile([64, 3 * NQ], f32, tag="g1")
            h1f = p_cd.tile([64, 3 * NQ], f16, tag="h1f")
            idxw = p_cd.tile([64, 3 * NQ // 16], i16, tag="idxw")

            # ---- phase C: gather + h1 ----
            with nc.named_scope("gather"):
                for g in range(4):
                    nc.sync.dma_start(idxw[16 * g:16 * (g + 1), :], w16d.bitcast(i16))
                nc.gpsimd.ap_gather(
                    out_ap=g1[:], in_ap=A[:], idxs_ap=idxw[:],
                    channels=64, num_elems=N, d=1, num_idxs=3 * NQ)
                bvb = Bv.unsqueeze(1).to_broadcast([64, 3, NQ])
                g13 = g1.rearrange("p (k q) -> p k q", k=3)
                nc.vector.tensor_add(g13, g13, bvb)
                nc.scalar.activation(h1f[:], g1[:], Relu,
                                     bias=sb1s[:, 1:2], scale=sb1s[:, 0:1])
            h1 = h1f.rearrange("p (k q) -> p k q", k=3)  # [64, 3, NQ] fp16

            # ---- phase D: convs (fp16 weights/acts, fp32 psum) ----
            with nc.named_scope("convs"):
                with tc.tile_pool(name="seg", bufs=2) as p_seg, \
                     tc.tile_pool(name="tmp", bufs=2) as p_tmp, \
                     tc.tile_pool(name="osb", bufs=2) as p_osb, \
                     tc.tile_pool(name="psd", bufs=4, space="PSUM") as p_psd:
                    outr = out.rearrange("(c p) n -> p c n", p=128)
                    for seg in range(NSEG):
                        qs = slice(seg * SEG, (seg + 1) * SEG)
                        h2 = p_seg.tile([128, 3, SEG], f16, tag="h2")
                        h3 = p_seg.tile([128, 2, 3, SEG], f16, tag="h3")
                        h4 = p_seg.tile([128, 4, 3, SEG], f16, tag="h4")
                        cat = p_seg.tile([128, 8, SEG], f16, tag="cat")
                        osb = p_osb.tile([128, 8, SEG], f32, tag="osb")
                        nc.vector.memset(cat[64:128, 0, :], 0.0)

                        # conv2 (K=64 -> 128)
                        for kk in range(3):
                            ps2 = p_psd.tile([128, SEG], f32, tag="psd")
                            nc.tensor.matmul(ps2[:], w2s[:], h1[:, kk, qs],
                                             start=True, stop=True)
                            nc.scalar.activation(h2[:, kk, :], ps2[:], Relu,
                                                 bias=sb2s[:, 1:2], scale=sb2s[:, 0:1])
                        # x1 -> cat chunk 0 (64 rows)
                        t1 = p_tmp.tile([64, SEG], f16, tag="t64")
                        nc.vector.tensor_max(t1[:], h1[:, 1, qs], h1[:, 2, qs])
                        nc.vector.tensor_max(cat[0:64, 0, :], t1[:], h1[:, 0, qs])
                        # x2 -> cat chunk 1
                        t2 = p_tmp.tile([128, SEG], f16, tag="t128")
                        nc.vector.tensor_max(t2[:], h2[:, 1, :], h2[:, 2, :])
                        nc.vector.tensor_max(cat[:, 1, :], t2[:], h2[:, 0, :])

                        # conv3 (K=128 -> 256 in 2 chunks)
                        for m in range(2):
                            for kk in range(3):
                                ps3 = p_psd.tile([128, SEG], f32, tag="psd")
                                nc.tensor.matmul(ps3[:], w3s[:, m * 128:(m + 1) * 128],
                                                 h2[:, kk, :], start=True, stop=True)
                                nc.scalar.activation(h3[:, m, kk, :], ps3[:], Relu,
                                                     bias=sb3s[:, 2 + m:3 + m],
                                                     scale=sb3s[:, m:m + 1])
                        # x3 -> cat chunks 2,3
                        for m in range(2):
                            t3 = p_tmp.tile([128, SEG], f16, tag="t128")
                            nc.vector.tensor_max(t3[:], h3[:, m, 1, :], h3[:, m, 2, :])
                            nc.vector.tensor_max(cat[:, 2 + m, :], t3[:], h3[:, m, 0, :])

                        # conv4 (K=256 in 2 chunks -> 512 in 4 chunks)
                        for m in range(4):
                            for kk in range(3):
                                ps4 = p_psd.tile([128, SEG], f32, tag="psd")
                                for c in range(2):
                                    nc.tensor.matmul(
                                        ps4[:], w4s[:, c, m * 128:(m + 1) * 128],
                                        h3[:, c, kk, :], start=(c == 0), stop=(c == 1))
                                nc.scalar.activation(h4[:, m, kk, :], ps4[:], Relu,
                                                     bias=sb4s[:, 4 + m:5 + m],
                                                     scale=sb4s[:, m:m + 1])
                        # x4 -> cat chunks 4..7
                        for m in range(4):
                            t4 = p_tmp.tile([128, SEG], f16, tag="t128")
                            nc.vector.tensor_max(t4[:], h4[:, m, 1, :], h4[:, m, 2, :])
                            nc.vector.tensor_max(cat[:, 4 + m, :], t4[:], h4[:, m, 0, :])

                        # conv5 (K=960 padded to 8*128 -> 1024 in 8 chunks)
                        for m in range(8):
                            ps5 = p_psd.tile([128, SEG], f32, tag="psd")
                            for c in range(8):
                                nc.tensor.matmul(
                                    ps5[:], w5s[:, c, m * 128:(m + 1) * 128],
                                    cat[:, c, :], start=(c == 0), stop=(c == 7))
                            nc.scalar.activation(osb[:, m, :], ps5[:], Relu,
                                                 bias=sb5s[:, 8 + m:9 + m],
                                                 scale=sb5s[:, m:m + 1])
                        nc.sync.dma_start(outr[:, :, qs], osb[:])


def prep_inputs(inputs):
    """Host-side sharding + layout/precision prep. Returns per-core in_maps."""
    import ml_dtypes

    x = np.ascontiguousarray(inputs["x"], dtype=np.float32)  # [B, C, N]
    shared = {}
    w1 = inputs["w1"].astype(np.float32)
    w1p = np.zeros((CPAD, 128), dtype=np.float16)
    w1p[:C_IN, 0:64] = w1[:, :C_IN].T.astype(np.float16)
    w1p[:C_IN, 64:128] = w1[:, C_IN:].T.astype(np.float16)
    shared["w1t"] = w1p
    shared["w2t"] = np.ascontiguousarray(inputs["w2"].T.astype(np.float16))
    shared["w3t"] = np.ascontiguousarray(inputs["w3"].T.astype(np.float16))
    shared["w4t"] = np.ascontiguousarray(inputs["w4"].T.astype(np.float16))
    w5t = inputs["w5"].astype(np.float32).T  # [960, 1024]
    w5p = np.zeros((128, 8, 1024), dtype=np.float16)
    w5p[0:64, 0, :] = w5t[0:64]          # x1 block
    w5p[:, 1, :] = w5t[64:192]           # x2
    w5p[:, 2, :] = w5t[192:320]          # x3 lo
    w5p[:, 3, :] = w5t[320:448]          # x3 hi
    for m in range(4):                   # x4
        w5p[:, 4 + m, :] = w5t[448 + 128 * m:448 + 128 * (m + 1)]
    shared["w5p"] = w5p

    def scale_bias(i):
        g = inputs[f"g{i}"].astype(np.float32)
        b = inputs[f"b{i}"].astype(np.float32)
        m = inputs[f"m{i}"].astype(np.float32)
        v = inputs[f"v{i}"].astype(np.float32)
        s = g / np.sqrt(v + EPS)
        return s.astype(np.float32), (b - m * s).astype(np.float32)

    s1, b1 = scale_bias(1)
    shared["sb1"] = np.ascontiguousarray(np.stack([s1, b1], axis=1))
    s2, b2 = scale_bias(2)
    shared["sb2"] = np.ascontiguousarray(np.stack([s2, b2], axis=1))
    s3, b3 = scale_bias(3)
    shared["sb3"] = np.ascontiguousarray(
        np.stack([s3[:128], s3[128:], b3[:128], b3[128:]], axis=1))
    s4, b4 = scale_bias(4)
    shared["sb4"] = np.ascontiguousarray(np.stack(
        [s4[128 * m:128 * (m + 1)] for m in range(4)]
        + [b4[128 * m:128 * (m + 1)] for m in range(4)], axis=1))
    s5, b5 = scale_bias(5)
    shared["sb5"] = np.ascontiguousarray(np.stack(
        [s5[128 * m:128 * (m + 1)] for m in range(8)]
        + [b5[128 * m:128 * (m + 1)] for m in range(8)], axis=1))

    in_maps = []
    for core in range(8):
        b, half = core // 2, core % 2
        q0 = half * NQ
        other0 = NQ - q0  # 2048 if half==0 else 0
        xbp = np.concatenate([x[b][:, q0:q0 + NQ], x[b][:, other0:other0 + NQ]],
                             axis=1)  # [1000, 4096], own queries first
        xpad = np.zeros((CPAD, N), dtype=np.float32)
        xpad[:C_IN] = xbp
        hi = xpad.astype(ml_dtypes.bfloat16)
        lo = (xpad - hi.astype(np.float32)).astype(ml_dtypes.bfloat16)
        m = dict(shared)
        m["xh"] = np.ascontiguousarray(hi)
        m["xl"] = np.ascontiguousarray(lo)
        in_maps.append(m)
    return in_maps


def kernel(**inputs):
    from concourse.bass_utils import run_bass_kernel_spmd

    if "nc" not in _CACHE:
        _CACHE["nc"] = build_nc()
    nc = _CACHE["nc"]
    in_maps = prep_inputs(inputs)
    res = run_bass_kernel_spmd(nc, in_maps, core_ids=list(range(8)))
    out = np.empty((B, 1024, N), dtype=np.float32)
    for core in range(8):
        b, half = core // 2, core % 2
        q0 = half * NQ
        out[b, :, q0:q0 + NQ] = res.results[core]["out"]
    return out
